# revision 1
# baseline (speedup 1.0000x reference)
"""Pairwise cosine-distance matrix kernel for Trainium2 (Bass/Tile, 8 cores).

Problem: mapping [8192, 512] fp32 -> out[i,j] = 1 - <x_i, x_j> / (|x_i||x_j|),
full [8192, 8192] fp32 output.

Strategy (SPMD over 8 NeuronCores):
  - Host passes each core the transposed matrix XT = mapping.T ([512, 8192],
    replicated) plus the core's own row-block transposed RT ([512, 1024]).
    Transposition is a pure layout choice: the PE contracts along the
    partition dim, so both matmul operands need the feature dim on partitions.
  - On device, each core computes column norms (square -> ones-matmul
    partition reduction in fp32 -> K=1 broadcast matmul -> reciprocal ->
    sqrt), normalizes columns into a bounce tile, and (default "b3" mode)
    splits each normalized fp32 value into a bf16 hi/lo pair stored in place
    of the fp32 data. The gram block [1024, 8192] is then computed as
    hi*hi + hi*lo + lo*hi (lo*lo ~ 2^-18 dropped) — three bf16 passes at
    1 PE cycle/row instead of fp32's 4 — accumulated in fp32 PSUM, with a
    fused (1 - x) epilogue split across ACT/DVE and 1 MiB output DMAs.
  - Host concatenates the 8 row blocks.
"""

import json
import os
import sys
import types

import numpy as np

N = 8192
D = 512
N_CORES = 8
ROWS_PER_CORE = N // N_CORES          # 1024
KC = D // 128                          # 4 k-chunks of 128
NT = N // 512                          # 16 column tiles of 512
NT_R = ROWS_PER_CORE // 512            # 2 column tiles of own block
MT = ROWS_PER_CORE // 128              # 8 row tiles of 128
NG = 4                                 # n-tiles per output staging group

# "b3": bf16 hi/lo 3-pass gram (fast, ~4e-6 abs err).
# "f32": exact fp32 gram (4 cycles/row, slower).
# "f32r": TF32-like single-pass (~1e-4 err) — experiment only.
GRAM_DT = os.environ.get("BASSKNN_GRAM_DT", "b3")

LAST_EXEC_NS = None  # max-across-traced-cores HW time of the last profiled run

_cached = {}


def _install_ntff_hook():
    """bass_utils' trace path imports antenv.axon_hooks, which this image
    lacks; recreate it and register the ctypes NTFF hook (same thing the
    boot script would have done)."""
    if "antenv.axon_hooks" in sys.modules:
        return
    mod = types.ModuleType("antenv.axon_hooks")
    holder = [None]
    mod.set_axon_ntff_profile_hook = lambda h: holder.__setitem__(0, h)
    mod.get_axon_ntff_profile_hook = lambda: holder[0]
    sys.modules["antenv.axon_hooks"] = mod
    import antenv
    antenv.axon_hooks = mod
    try:
        from trn_agent_boot.trn_boot import _ntff_profile_via_ctypes
        mod.set_axon_ntff_profile_hook(
            _ntff_profile_via_ctypes("/opt/axon/libaxon_pjrt.so")
        )
    except Exception:
        pass


def _split_multiwait_bir(bir_json: bytes) -> bytes:
    """This container's walrus rejects instructions with >1 semaphore wait
    ("Too many sync wait commands"). Hoist extra waits onto standalone
    wait-only EventSemaphore instructions placed just before, on the same
    engine — identical stall semantics."""
    m = json.loads(bir_json)
    for f in m["functions"]:
        for bb in f.get("blocks", f.get("basicblocks", [])):
            new_insts = []
            for inst in bb["instructions"]:
                si = inst.get("sync_info")
                waits = si.get("on_wait") if si else None
                if waits and len(waits) > 1:
                    for j, w in enumerate(waits[:-1]):
                        new_insts.append({
                            "debug": inst.get("debug"),
                            "engine": inst["engine"],
                            "ins": [],
                            "name": f"{inst['name']}-hw{j}",
                            "opcode": "EventSemaphore",
                            "outs": [],
                            "sync_info": {"on_update": [], "on_wait": [w]},
                        })
                    si["on_wait"] = [waits[-1]]
                new_insts.append(inst)
            bb["instructions"] = new_insts
    return json.dumps(m).encode()


def _apply_patches():
    if _cached.get("patched"):
        return
    _cached["patched"] = True
    import concourse.bass2jax as bass2jax
    import concourse.bass_utils as bass_utils

    orig_compile = bass2jax.compile_bir_kernel

    def patched_compile(bir_json, tmpdir, neff_name="file.neff"):
        return orig_compile(_split_multiwait_bir(bir_json), tmpdir,
                            neff_name=neff_name)

    bass2jax.compile_bir_kernel = patched_compile
    # No S3 in this container; the trace path uploads artifacts for links only.
    bass_utils.upload_artifacts = lambda tmpdir: "local://" + tmpdir


def _build(gram_dt: str):
    key = ("nc", gram_dt)
    if key in _cached:
        return _cached[key]
    _apply_patches()
    import concourse.bass as bass
    import concourse.tile as tile
    from concourse import mybir

    f32 = mybir.dt.float32
    f32r = mybir.dt.float32r
    bf16 = mybir.dt.bfloat16
    Act = mybir.ActivationFunctionType
    Alu = mybir.AluOpType

    nc = bass.Bass(trn_type="TRN2", target_bir_lowering=False, debug=False)
    xt_d = nc.dram_tensor("xt", [D, N], f32, kind="ExternalInput").ap()
    rt_d = nc.dram_tensor("rt", [D, ROWS_PER_CORE], f32, kind="ExternalInput").ap()
    out_d = nc.dram_tensor("out", [ROWS_PER_CORE, N], f32, kind="ExternalOutput").ap()

    with tile.TileContext(nc) as tc:
        with (
            tc.tile_pool(name="xt", bufs=1) as xt_pool,
            tc.tile_pool(name="rt", bufs=1) as rt_pool,
            tc.tile_pool(name="tmp", bufs=8) as tmp_pool,
            tc.tile_pool(name="sq", bufs=5) as sq_pool,
            tc.tile_pool(name="rows", bufs=2) as row_pool,
            tc.tile_pool(name="consts", bufs=1) as const_pool,
            tc.tile_pool(name="stage", bufs=2) as stage_pool,
            tc.tile_pool(name="ps_n2bc", bufs=2, space=bass.MemorySpace.PSUM) as ps_n2bc,
            tc.tile_pool(name="ps_g", bufs=6, space=bass.MemorySpace.PSUM) as ps_g,
        ):
            ones_col = const_pool.tile([128, 1], f32, name="ones_col")
            nc.vector.memset(ones_col[:], 1.0)
            ones_row = const_pool.tile([1, 128], f32, name="ones_row")
            nc.vector.memset(ones_row[:], 1.0)
            one_bias = const_pool.tile([128, 1], f32, name="one_bias")
            nc.vector.memset(one_bias[:], 1.0)

            xt = [xt_pool.tile([128, N], f32, tag=f"xt{k}", name=f"xt{k}")
                  for k in range(KC)]
            rt = [rt_pool.tile([128, ROWS_PER_CORE], f32, tag=f"rt{k}",
                               name=f"rt{k}") for k in range(KC)]
            # rt first (gates everything), then xt group-major so group 0's
            # columns land first and the gram can start ~40us earlier.
            for k in range(KC):
                nc.sync.dma_start(out=rt[k][:], in_=rt_d[k * 128:(k + 1) * 128, :])
            gw = NG * 512
            for g in range(NT // NG):
                for k in range(KC):
                    nc.sync.dma_start(
                        out=xt[k][:, g * gw:(g + 1) * gw],
                        in_=xt_d[k * 128:(k + 1) * 128, g * gw:(g + 1) * gw])

            # bf16 views for hi/lo storage (mode b3): slice t of the fp32
            # tile ([2048t, 2048t+2048) bytes) is reused as hi at bf16
            # elements [1024t, 1024t+512) and lo at [1024t+512, 1024t+1024).
            xt_b = [t[:].bitcast(bf16) for t in xt]
            rt_b = [t[:].bitcast(bf16) for t in rt]

            def normalize(tiles, tiles_b, ts_, label):
                """Column-normalize the 512-wide slices ts_ of `tiles` (KC
                chunks): per slice, sum the squared chunks elementwise and
                partition-reduce with one fp32 ones-matmul; then one batched
                row reciprocal+sqrt for the whole slice group; then per
                slice a K=1 broadcast matmul and per-mode rescale/split."""
                nslc = len(ts_)
                grow = row_pool.tile([1, 512 * nslc], f32, tag="grow",
                                     name=f"grow_{label}{ts_[0]}")
                for i, t in enumerate(ts_):
                    sl = slice(t * 512, (t + 1) * 512)
                    sqs = []
                    for k in range(KC):
                        sq = sq_pool.tile([128, 512], f32, tag="sq",
                                          name=f"sq_{label}{t}_{k}")
                        nc.scalar.square(sq[:], tiles[k][:, sl])
                        sqs.append(sq)
                    # elementwise-sum the 4 chunks so the fp32 partition-
                    # reduce matmul (4 PE cycles/row) runs once, not 4 times
                    a01 = tmp_pool.tile([128, 512], f32, tag="tmp",
                                        name=f"a01_{label}{t}")
                    nc.vector.tensor_add(a01[:], sqs[0][:], sqs[1][:])
                    a23 = tmp_pool.tile([128, 512], f32, tag="tmp",
                                        name=f"a23_{label}{t}")
                    nc.vector.tensor_add(a23[:], sqs[2][:], sqs[3][:])
                    ssum = tmp_pool.tile([128, 512], f32, tag="tmp",
                                         name=f"ssum_{label}{t}")
                    nc.vector.tensor_add(ssum[:], a01[:], a23[:])
                    n2 = ps_n2bc.tile([1, 512], f32, tag="n2bc",
                                      name=f"n2_{label}{t}")
                    nc.tensor.matmul(n2[:], ones_col[:], ssum[:],
                                     start=True, stop=True)
                    nc.scalar.copy(grow[0:1, i * 512:(i + 1) * 512], n2[:])
                # one batched rsqrt for the whole group: 1/sqrt on the row
                grn = row_pool.tile([1, 512 * nslc], f32, tag="grow",
                                    name=f"grn_{label}{ts_[0]}")
                nc.vector.reciprocal(grn[:], grow[:])
                nc.scalar.sqrt(grn[:], grn[:])
                for i, t in enumerate(ts_):
                    sl = slice(t * 512, (t + 1) * 512)
                    bc = ps_n2bc.tile([128, 512], f32, tag="n2bc",
                                      name=f"bc_{label}{t}")
                    nc.tensor.matmul(bc[:], ones_row[:],
                                     grn[0:1, i * 512:(i + 1) * 512],
                                     start=True, stop=True)
                    for k in range(KC):
                        if gram_dt == "b3":
                            tmp = tmp_pool.tile([128, 512], f32, tag="tmp",
                                                name=f"tmp_{label}{t}_{k}")
                            # bc is PSUM: GpSimd has no PSUM access, keep DVE
                            nc.vector.tensor_mul(tmp[:], tiles[k][:, sl], bc[:])
                            hi = tiles_b[k][:, 1024 * t:1024 * t + 512]
                            lo = tiles_b[k][:, 1024 * t + 512:1024 * (t + 1)]
                            nc.scalar.copy(hi, tmp[:])
                            nc.vector.tensor_sub(lo, tmp[:], hi)
                        elif gram_dt == "f32r":
                            nc.vector.tensor_mul(tiles[k][:, sl].bitcast(f32r),
                                                 tiles[k][:, sl], bc[:])
                        else:
                            nc.vector.tensor_mul(tiles[k][:, sl],
                                                 tiles[k][:, sl], bc[:])

            def rt_w(k, mt, part):
                """Weight slice (lhsT [128, 128]) for row-tile mt, chunk k."""
                if gram_dt == "b3":
                    base = 1024 * (mt // 4) + (512 if part == "lo" else 0)
                    return rt_b[k][:, base + (mt % 4) * 128:
                                   base + (mt % 4) * 128 + 128]
                ap = rt[k][:, mt * 128:(mt + 1) * 128]
                return ap.bitcast(f32r) if gram_dt == "f32r" else ap

            def xt_m(k, nt, part):
                """Moving slice (rhs [128, 512]) for col-tile nt, chunk k."""
                if gram_dt == "b3":
                    base = 1024 * nt + (512 if part == "lo" else 0)
                    return xt_b[k][:, base:base + 512]
                ap = xt[k][:, nt * 512:(nt + 1) * 512]
                return ap.bitcast(f32r) if gram_dt == "f32r" else ap

            passes = ([("hi", "hi"), ("hi", "lo"), ("lo", "hi")]
                      if gram_dt == "b3" else [("hi", "hi")])

            normalize(rt, rt_b, list(range(NT_R)), "r")
            normalize(xt, xt_b, list(range(NG)), "x")

            for g in range(NT // NG):
                for mt in range(MT):
                    # prefetch next group's normalization into this group's
                    # gram stream so its chains hide under ~70us of matmuls;
                    # high_priority floats it as early as deps allow
                    if mt == 1 and g + 1 < NT // NG:
                        with tc.high_priority():
                            normalize(xt, xt_b,
                                      [(g + 1) * NG + j for j in range(NG)],
                                      "x")
                    psums = [ps_g.tile([128, 512], f32, tag="pg",
                                       name=f"pg_{g}_{mt}_{j}")
                             for j in range(NG)]
                    n_acc = len(passes) * KC
                    acc = 0
                    for wp, mp in passes:
                        for k in range(KC):
                            for j in range(NG):
                                nc.tensor.matmul(psums[j][:],
                                                 rt_w(k, mt, wp),
                                                 xt_m(k, g * NG + j, mp),
                                                 start=(acc == 0),
                                                 stop=(acc == n_acc - 1))
                            acc += 1
                    stage = stage_pool.tile([128, NG * 512], f32, tag="st",
                                            name=f"st_{g}_{mt}")
                    for j in range(NG):
                        ssl = slice(j * 512, (j + 1) * 512)
                        if j % 2 == 0:
                            nc.scalar.activation(stage[:, ssl], psums[j][:],
                                                 Act.Identity,
                                                 bias=one_bias[:], scale=-1.0)
                        else:
                            nc.vector.tensor_scalar(stage[:, ssl], psums[j][:],
                                                    -1.0, 1.0,
                                                    Alu.mult, Alu.add)
                    nc.sync.dma_start(
                        out=out_d[mt * 128:(mt + 1) * 128,
                                  g * NG * 512:(g + 1) * NG * 512],
                        in_=stage[:])

    _cached[key] = nc
    return nc


def kernel(mapping: np.ndarray) -> np.ndarray:
    from concourse.bass_utils import run_bass_kernel_spmd

    mapping = np.ascontiguousarray(mapping, dtype=np.float32)
    assert mapping.shape == (N, D)
    xt = np.ascontiguousarray(mapping.T)
    in_maps = []
    for c in range(N_CORES):
        rt = np.ascontiguousarray(xt[:, c * ROWS_PER_CORE:(c + 1) * ROWS_PER_CORE])
        in_maps.append({"xt": xt, "rt": rt})

    nc = _build(GRAM_DT)

    trace = bool(int(os.environ.get("BASSKNN_TRACE", "0")))
    if trace:
        _install_ntff_hook()
    res = run_bass_kernel_spmd(nc, in_maps, list(range(N_CORES)), trace=trace)
    global LAST_EXEC_NS
    if trace:
        LAST_EXEC_NS = res.exec_time_ns

    out = np.concatenate([res.results[c]["out"] for c in range(N_CORES)], axis=0)
    return out.astype(np.float32, copy=False)



# revision 4
# speedup vs baseline: 3.2657x; 3.2657x over previous
"""Pairwise cosine-distance matrix kernel for Trainium2 (Bass/Tile, 8 cores).

Problem: mapping [8192, 512] fp32 -> out[i,j] = 1 - <x_i, x_j> / (|x_i||x_j|),
full [8192, 8192] fp32 output.

Strategy (SPMD over 8 NeuronCores, symmetric-triangle partitioning):
  - The output is symmetric, so only the 136 distinct [512, 512] blocks of
    the 16x16 block grid need device compute. Circulant assignment: row-block
    r computes blocks (r, r+d mod 16) for d = 0..7, and the 8 "bridge"
    blocks (c, c+8) go one per core. Core c owns row-blocks c and c+8 ->
    exactly 17 blocks per core, with a uniform structure (weight A x 9
    column tiles + weight B x 8 column tiles) so a single NEFF serves all
    cores SPMD.
  - The host rotates the transposed matrix's columns by 512*c per core so
    that each core's weight/moving slices sit at identical compile-time
    offsets. Host input is fp16 (halves input DMA; the 2e-2 rel-err budget
    dwarfs fp16 quantization).
  - On device: per 512-column tile, compute column norms (ACT square ->
    DVE elementwise-sum of the 4 k-chunks -> ones-matmul partition reduce
    -> ACT rsqrt -> K=1 broadcast matmul) and scale the tile in place
    (fp16). The gram blocks run as fp16 matmuls (1 PE cycle/row) with fp32
    PSUM accumulation and a fused (1 - x) epilogue split across ACT/DVE
    into fp16 staging tiles, then DMA out.
  - Host upcasts to fp32, places each block, and mirrors its transpose to
    the symmetric position.
"""

import json
import os
import sys
import types

import numpy as np

N = 8192
D = 512
N_CORES = 8
NB = 16                 # 512-wide row/col blocks
BS = N // NB            # 512
KC = D // 128           # 4 k-chunks of 128
MT = BS // 128          # 4 row-chunks of 128 per 512-row part

# tiles normalized together; group g gates gram group g
NORM_GROUPS = [[0, 1, 2], [3, 4, 5], [6, 7, 8], [9, 10, 11], [12, 13, 14, 15]]
# (weight_tile, moving_tiles, out_name, out_col_offset)
GRAM_GROUPS = [
    (0, [0, 1, 2], "outA", 0),
    (0, [3, 4, 5], "outA", 3 * BS),
    (0, [6, 7, 8], "outA", 6 * BS),
    (8, [8, 9, 10, 11], "outB", 0),
    (8, [12, 13, 14, 15], "outB", 4 * BS),
]

LAST_EXEC_NS = None  # max-across-traced-cores HW time of the last profiled run

_cached = {}


def _install_ntff_hook():
    """bass_utils' trace path imports antenv.axon_hooks, which this image
    lacks; recreate it and register the ctypes NTFF hook (same thing the
    boot script would have done)."""
    if "antenv.axon_hooks" in sys.modules:
        return
    mod = types.ModuleType("antenv.axon_hooks")
    holder = [None]
    mod.set_axon_ntff_profile_hook = lambda h: holder.__setitem__(0, h)
    mod.get_axon_ntff_profile_hook = lambda: holder[0]
    sys.modules["antenv.axon_hooks"] = mod
    import antenv
    antenv.axon_hooks = mod
    try:
        from trn_agent_boot.trn_boot import _ntff_profile_via_ctypes
        mod.set_axon_ntff_profile_hook(
            _ntff_profile_via_ctypes("/opt/axon/libaxon_pjrt.so")
        )
    except Exception:
        pass


def _split_multiwait_bir(bir_json: bytes) -> bytes:
    """This container's walrus rejects instructions with >1 semaphore wait
    ("Too many sync wait commands"). Hoist extra waits onto standalone
    wait-only EventSemaphore instructions placed just before, on the same
    engine — identical stall semantics."""
    m = json.loads(bir_json)
    for f in m["functions"]:
        for bb in f.get("blocks", f.get("basicblocks", [])):
            new_insts = []
            for inst in bb["instructions"]:
                si = inst.get("sync_info")
                waits = si.get("on_wait") if si else None
                if waits and len(waits) > 1:
                    for j, w in enumerate(waits[:-1]):
                        new_insts.append({
                            "debug": inst.get("debug"),
                            "engine": inst["engine"],
                            "ins": [],
                            "name": f"{inst['name']}-hw{j}",
                            "opcode": "EventSemaphore",
                            "outs": [],
                            "sync_info": {"on_update": [], "on_wait": [w]},
                        })
                    si["on_wait"] = [waits[-1]]
                new_insts.append(inst)
            bb["instructions"] = new_insts
    return json.dumps(m).encode()


def _apply_patches():
    if _cached.get("patched"):
        return
    _cached["patched"] = True
    import concourse.bass2jax as bass2jax
    import concourse.bass_utils as bass_utils

    orig_compile = bass2jax.compile_bir_kernel

    def patched_compile(bir_json, tmpdir, neff_name="file.neff"):
        return orig_compile(_split_multiwait_bir(bir_json), tmpdir,
                            neff_name=neff_name)

    bass2jax.compile_bir_kernel = patched_compile
    # No S3 in this container; the trace path uploads artifacts for links only.
    bass_utils.upload_artifacts = lambda tmpdir: "local://" + tmpdir


def _build():
    key = "nc"
    if key in _cached:
        return _cached[key]
    _apply_patches()
    import concourse.bass as bass
    import concourse.tile as tile
    from concourse import mybir

    f32 = mybir.dt.float32
    f16 = mybir.dt.float16
    Act = mybir.ActivationFunctionType
    Alu = mybir.AluOpType

    nc = bass.Bass(trn_type="TRN2", target_bir_lowering=False, debug=False)
    xt_d = nc.dram_tensor("xt", [D, N], f16, kind="ExternalInput").ap()
    outA_d = nc.dram_tensor("outA", [BS, 9 * BS], f16, kind="ExternalOutput").ap()
    outB_d = nc.dram_tensor("outB", [BS, 8 * BS], f16, kind="ExternalOutput").ap()

    with tile.TileContext(nc) as tc:
        with (
            tc.tile_pool(name="xt", bufs=1) as xt_pool,
            tc.tile_pool(name="sq", bufs=5) as sq_pool,
            tc.tile_pool(name="tmp", bufs=6) as tmp_pool,
            tc.tile_pool(name="rows", bufs=4) as row_pool,
            tc.tile_pool(name="consts", bufs=1) as const_pool,
            tc.tile_pool(name="stage", bufs=3) as stage_pool,
            tc.tile_pool(name="ps_nb", bufs=2, space=bass.MemorySpace.PSUM) as ps_nb,
            tc.tile_pool(name="ps_g", bufs=6, space=bass.MemorySpace.PSUM) as ps_g,
        ):
            ones_col = const_pool.tile([128, 1], f16, name="ones_col")
            nc.vector.memset(ones_col[:], 1.0)
            ones_row = const_pool.tile([1, 128], f16, name="ones_row")
            nc.vector.memset(ones_row[:], 1.0)
            one_bias = const_pool.tile([128, 1], f32, name="one_bias")
            nc.vector.memset(one_bias[:], 1.0)

            xt = [xt_pool.tile([128, N], f16, tag=f"xt{k}", name=f"xt{k}")
                  for k in range(KC)]
            # group-major input DMA so gram group 0's tiles land first
            for tiles in NORM_GROUPS:
                lo, hi = tiles[0] * BS, (tiles[-1] + 1) * BS
                for k in range(KC):
                    nc.sync.dma_start(out=xt[k][:, lo:hi],
                                      in_=xt_d[k * 128:(k + 1) * 128, lo:hi])

            def normalize(tiles):
                """Column-normalize the 512-wide slices `tiles` of xt in
                place (fp16): square -> sum k-chunks -> ones-matmul
                partition reduce -> rsqrt -> K=1 broadcast matmul ->
                elementwise scale."""
                for t in tiles:
                    sl = slice(t * BS, (t + 1) * BS)
                    sqs = []
                    for k in range(KC):
                        sq = sq_pool.tile([128, BS], f32, tag="sq",
                                          name=f"sq{t}_{k}")
                        nc.scalar.square(sq[:], xt[k][:, sl])
                        sqs.append(sq)
                    a01 = tmp_pool.tile([128, BS], f32, tag="tmp",
                                        name=f"a01_{t}")
                    nc.vector.tensor_add(a01[:], sqs[0][:], sqs[1][:])
                    a23 = tmp_pool.tile([128, BS], f32, tag="tmp",
                                        name=f"a23_{t}")
                    nc.vector.tensor_add(a23[:], sqs[2][:], sqs[3][:])
                    ssum = tmp_pool.tile([128, BS], f16, tag="tmp",
                                         name=f"ssum_{t}")
                    nc.vector.tensor_add(ssum[:], a01[:], a23[:])
                    n2 = ps_nb.tile([1, BS], f32, tag="nb", name=f"n2_{t}")
                    nc.tensor.matmul(n2[:], ones_col[:], ssum[:],
                                     start=True, stop=True)
                    # rsqrt = exp(-0.5*ln(x)): ACT Rsqrt/Reciprocal are
                    # blocked for accuracy, and DVE reciprocal is ~6.3
                    # ns/elem on a single-partition row (iterative)
                    lnx = row_pool.tile([1, BS], f32, tag="rn",
                                        name=f"ln_{t}")
                    nc.scalar.activation(lnx[:], n2[:], Act.Ln)
                    rn = row_pool.tile([1, BS], f16, tag="rn", name=f"rn_{t}")
                    nc.scalar.activation(rn[:], lnx[:], Act.Exp, scale=-0.5)
                    bc = ps_nb.tile([128, BS], f32, tag="nb", name=f"bc_{t}")
                    nc.tensor.matmul(bc[:], ones_row[:], rn[:],
                                     start=True, stop=True)
                    for k in range(KC):
                        nc.vector.tensor_mul(xt[k][:, sl], xt[k][:, sl], bc[:])

            normalize(NORM_GROUPS[0])

            for gi, (wt, tiles, out_name, off) in enumerate(GRAM_GROUPS):
                out_d = outA_d if out_name == "outA" else outB_d
                nt = len(tiles)
                for mt in range(MT):
                    # prefetch the next norm group under this group's gram
                    # stream; high_priority floats it as early as deps allow
                    if mt == 1 and gi + 1 < len(NORM_GROUPS):
                        with tc.high_priority():
                            normalize(NORM_GROUPS[gi + 1])
                    psums = [ps_g.tile([128, BS], f32, tag="pg",
                                       name=f"pg_{gi}_{mt}_{j}")
                             for j in range(nt)]
                    for k in range(KC):
                        w = xt[k][:, wt * BS + mt * 128:wt * BS + mt * 128 + 128]
                        for j, t in enumerate(tiles):
                            nc.tensor.matmul(psums[j][:], w,
                                             xt[k][:, t * BS:(t + 1) * BS],
                                             start=(k == 0), stop=(k == KC - 1))
                    stage = stage_pool.tile([128, nt * BS], f16, tag="st",
                                            name=f"st_{gi}_{mt}")
                    for j in range(nt):
                        ssl = slice(j * BS, (j + 1) * BS)
                        if j % 2 == 0:
                            nc.vector.tensor_scalar(stage[:, ssl], psums[j][:],
                                                    -1.0, 1.0,
                                                    Alu.mult, Alu.add)
                        else:
                            nc.scalar.activation(stage[:, ssl], psums[j][:],
                                                 Act.Identity,
                                                 bias=one_bias[:], scale=-1.0)
                    nc.sync.dma_start(
                        out=out_d[mt * 128:(mt + 1) * 128, off:off + nt * BS],
                        in_=stage[:])

    _cached[key] = nc
    return nc


def kernel(mapping: np.ndarray) -> np.ndarray:
    from concourse.bass_utils import run_bass_kernel_spmd

    mapping = np.ascontiguousarray(mapping, dtype=np.float32)
    assert mapping.shape == (N, D)
    xt16 = np.ascontiguousarray(mapping.T.astype(np.float16))  # [512, 8192]
    in_maps = []
    for c in range(N_CORES):
        in_maps.append({"xt": np.ascontiguousarray(
            np.roll(xt16, -BS * c, axis=1))})

    nc = _build()

    trace = bool(int(os.environ.get("BASSKNN_TRACE", "0")))
    if trace:
        _install_ntff_hook()
    res = run_bass_kernel_spmd(nc, in_maps, list(range(N_CORES)), trace=trace)
    global LAST_EXEC_NS
    if trace:
        LAST_EXEC_NS = res.exec_time_ns

    full = np.empty((N, N), np.float32)
    for c in range(N_CORES):
        A = np.asarray(res.results[c]["outA"]).astype(np.float32)
        B = np.asarray(res.results[c]["outB"]).astype(np.float32)
        for t in range(9):
            j = (c + t) % NB
            blk = A[:, t * BS:(t + 1) * BS]
            full[c * BS:(c + 1) * BS, j * BS:(j + 1) * BS] = blk
            if t:
                full[j * BS:(j + 1) * BS, c * BS:(c + 1) * BS] = blk.T
        i2 = c + 8
        for e in range(8):
            j = (i2 + e) % NB
            blk = B[:, e * BS:(e + 1) * BS]
            full[i2 * BS:(i2 + 1) * BS, j * BS:(j + 1) * BS] = blk
            if e:
                full[j * BS:(j + 1) * BS, i2 * BS:(i2 + 1) * BS] = blk.T
    return full
